# revision 5
# baseline (speedup 1.0000x reference)
"""Segment-prefix max kernel for Trainium2 (8 NeuronCores, SPMD).

Problem: x [1048576, 128] f32, 2048 uniform segments of 512 rows each;
out[i, :] = max over the first (512 - window_size + 1) rows of segment i.

Strategy (memory-bound, ~512 MiB streamed from HBM):
  - Shard segments across 8 cores: core c gets rows [c*131072, (c+1)*131072)
    and produces out rows [c*256, (c+1)*256). No cross-core communication.
  - Per core, each 512-row segment is loaded so SBUF partition p holds rows
    {4p..4p+3} of the segment (2 KiB contiguous DRAM runs); 2 MiB tiles of
    8 segments, alternating the SP and Activation HWDGE rings. The final
    tiles are small (2 segments) so the post-stream latency is short.
  - Three DVE tensor_max ops fold the 4 rows per partition down to 1,
    excluding the window's last rows via partition-sliced operands.
  - The cross-partition max (128 -> 1) runs through PE transposes (identity
    matmul), 4 segments into one PSUM bank, then a single DVE reduce_max
    along the free axis yields 4 output columns at once.
  - Output columns accumulate in [128, 64] SBUF chunks that are
    PE-transposed back to row-major and DMA'd out every 64 segments, so
    stores overlap the stream instead of serializing at the end.
"""

import numpy as np

import concourse.bacc as bacc
import concourse.bass as bass
import concourse.tile as tile
from concourse import mybir
from concourse.bass_utils import run_bass_kernel_spmd
from concourse.masks import make_identity

N_CORES = 8
SEG_LEN = 512
D = 128
J = 4  # segment rows stacked per partition (J * 128 partitions = 512 rows)
SEGS_PER_TILE = 8  # 8 segments * 512 rows * 128 * 4 B = 2 MiB per DMA load
CHUNK = 64  # output segments per flush
TAIL_TILES = 8  # number of small (2-seg) tiles at the end
TAIL_SEGS = 2

_PROGRAM_CACHE: dict = {}


def _build_program(n_seg_core: int, count: int) -> bacc.Bacc:
    """Bass program for one core: n_seg_core segments, max over first
    `count` rows of each."""
    rows = n_seg_core * SEG_LEN
    f32 = mybir.dt.float32

    # tile schedule: big tiles, then small tail tiles for a short endgame
    tail_segs_total = TAIL_TILES * TAIL_SEGS
    n_big = (n_seg_core - tail_segs_total) // SEGS_PER_TILE
    tiles = [SEGS_PER_TILE] * n_big + [TAIL_SEGS] * TAIL_TILES
    assert sum(tiles) == n_seg_core

    nc = bacc.Bacc("TRN2", target_bir_lowering=False, debug=False)
    x_in = nc.dram_tensor("x", [rows, D], f32, kind="ExternalInput")
    out_t = nc.dram_tensor("out", [n_seg_core, D], f32, kind="ExternalOutput")

    # row = ((seg*128 + p)*J + j ; partition p holds rows 4p..4p+3 of seg
    x_v = x_in.rearrange("(s p j) d -> s p j d", p=128, j=J)

    # valid partitions for j-view: rows J*p + j < count
    v = [max(0, min(128, (count - j + J - 1) // J)) if count > j else 0 for j in range(J)]
    fast = v[0] == 128 and v[1] == 128  # rows 4p, 4p+1 valid everywhere

    with tile.TileContext(nc) as tc:
        with (
            tc.tile_pool(name="io", bufs=8) as io_pool,
            tc.tile_pool(name="work", bufs=4) as work_pool,
            tc.tile_pool(name="och", bufs=2) as och_pool,
            tc.tile_pool(name="ot", bufs=2) as ot_pool,
            tc.tile_pool(name="psum", bufs=6, space="PSUM") as psum_pool,
            tc.tile_pool(name="pso", bufs=2, space="PSUM") as pso_pool,
            tc.tile_pool(name="consts", bufs=1) as consts,
        ):
            ident = consts.tile([128, 128], f32)
            make_identity(nc, ident)

            outchunk = None
            seg0 = 0
            for t, S in enumerate(tiles):
                if seg0 % CHUNK == 0:
                    outchunk = och_pool.tile([128, CHUNK], f32, tag="och")

                tl = io_pool.tile([128, S, J, D], f32, tag=f"tl{S}")
                hw = nc.sync if t % 2 == 0 else nc.scalar
                hw.dma_start(
                    out=tl,
                    in_=x_v[seg0 : seg0 + S].rearrange("s p j d -> p s j d"),
                )

                acc = work_pool.tile([128, S, D], f32, tag=f"acc{S}")
                if fast:
                    nc.vector.tensor_max(
                        out=acc, in0=tl[:, :, 0, :], in1=tl[:, :, 1, :]
                    )
                    for j in range(2, J):
                        if v[j] > 0:
                            nc.vector.tensor_max(
                                out=acc[: v[j]],
                                in0=acc[: v[j]],
                                in1=tl[: v[j], :, j, :],
                            )
                else:
                    nc.vector.memset(acc, float("-inf"))
                    for j in range(J):
                        if v[j] > 0:
                            nc.vector.tensor_max(
                                out=acc[: v[j]],
                                in0=acc[: v[j]],
                                in1=tl[: v[j], :, j, :],
                            )

                for g in range(0, S, 4):
                    gs = min(4, S - g)
                    bank = psum_pool.tile([128, 4, 128], f32, tag="pt")
                    for c in range(gs):
                        nc.tensor.transpose(
                            bank[:, c, :], acc[:, g + c, :], ident
                        )
                    co = (seg0 + g) % CHUNK
                    nc.vector.reduce_max(
                        out=outchunk[:, co : co + gs], in_=bank[:, 0:gs, :],
                        axis=mybir.AxisListType.X,
                    )

                seg0 += S
                if seg0 % CHUNK == 0:
                    m = seg0 // CHUNK - 1
                    pt = pso_pool.tile([CHUNK, 128], f32, tag="ptout")
                    nc.tensor.transpose(pt, outchunk, ident)
                    ot = ot_pool.tile([CHUNK, 128], f32, tag="ot")
                    nc.scalar.copy(ot, pt)
                    nc.scalar.dma_start(
                        out=out_t[m * CHUNK : (m + 1) * CHUNK, :], in_=ot
                    )
    nc.compile()
    return nc


def _numpy_fallback(x: np.ndarray, sizes: np.ndarray, w: int) -> np.ndarray:
    ends = np.cumsum(sizes)
    starts = ends - sizes
    out = np.full((sizes.shape[0], x.shape[1]), -np.inf, dtype=np.float32)
    for i in range(sizes.shape[0]):
        c = int(sizes[i]) - w + 1
        if c > 0:
            out[i] = x[int(starts[i]) : int(starts[i]) + c].max(axis=0)
    return out


def kernel(x, sizes, window_size) -> np.ndarray:
    x = np.ascontiguousarray(np.asarray(x, dtype=np.float32))
    sizes = np.asarray(sizes)
    w = int(np.asarray(window_size))
    n_seg = sizes.shape[0]
    count = SEG_LEN - w + 1

    n_seg_core = n_seg // N_CORES if n_seg % N_CORES == 0 else 0
    uniform = (
        x.ndim == 2
        and x.shape[1] == D
        and bool((sizes == SEG_LEN).all())
        and x.shape[0] == n_seg * SEG_LEN
        and n_seg_core > 0
        and n_seg_core % CHUNK == 0
        and (n_seg_core - TAIL_TILES * TAIL_SEGS) % SEGS_PER_TILE == 0
        and n_seg_core >= TAIL_TILES * TAIL_SEGS + SEGS_PER_TILE
        and 0 < count <= SEG_LEN
    )
    if not uniform:
        return _numpy_fallback(x, sizes, w)

    key = (n_seg_core, count)
    if key not in _PROGRAM_CACHE:
        _PROGRAM_CACHE[key] = _build_program(n_seg_core, count)
    nc = _PROGRAM_CACHE[key]

    shards = np.split(x, N_CORES, axis=0)
    in_maps = [{"x": s} for s in shards]
    res = run_bass_kernel_spmd(nc, in_maps, core_ids=list(range(N_CORES)))
    return np.concatenate([r["out"] for r in res.results], axis=0)
